# revision 7
# baseline (speedup 1.0000x reference)
"""Trainium2 Bass kernel for NT-Xent contrastive loss (N=4096, D=256).

loss = mean_i(log(sum_{k!=i} exp(sim(r_i,r_k)/T)) - sim(r_i, r_{i+N mod 2N})/T)
with r = row-l2-normalized concat(emb_i, emb_j), T = 0.5.

Sharding: rows of the [8192, 8192] similarity matrix are split across the
8 cores (1024 rows each, passed per-core as `my_rows`). Every core builds
the full normalized transposed reps [256, 8192] (bf16) in SBUF via DMA
xbar transposes, computes its row-block of the Gram matrix on the PE in
[128, 2048] psum tiles, does exp+row-sum on the Scalar engine (fused
accumulator), excludes the diagonal analytically (exp(2*||rho_r||^2)),
takes one batched log, and reduces. Ln/Exp activations are batched so the
ACT table set never thrashes. The positive term is computed from
normalized row pairs on the Vector engine (identical on every core; each
core subtracts 1/8 of it). Host sums the 8 [128, 2] partials.
"""

import os
import numpy as np

import concourse.bass as bass
import concourse.bacc as bacc
import concourse.tile as tile
from concourse import mybir
from concourse.bass_utils import run_bass_kernel_spmd
from contextlib import ExitStack

N = 4096
D = 256
TWO_N = 2 * N
N_CORES = 8
ROWS_PER_CORE = TWO_N // N_CORES  # 1024
M_TILES = ROWS_PER_CORE // 128    # 8
FULL_TILES = TWO_N // 128         # 64 (32 from emb_i, 32 from emb_j)
KC = 2                            # 256 = 2 chunks of 128 on partitions

F32 = mybir.dt.float32
BF16 = mybir.dt.bfloat16
ALU = mybir.AluOpType
ACT = mybir.ActivationFunctionType
AXX = mybir.AxisListType


def _emit(nc, tc, ctx, emb_i, emb_j, my_rows, out):
    persist = ctx.enter_context(tc.tile_pool(name="persist", bufs=1))
    work = ctx.enter_context(tc.tile_pool(name="work", bufs=3))
    psum_mm = ctx.enter_context(tc.tile_pool(name="psum_mm", bufs=2, space="PSUM"))

    # ---- persistent SBUF ----
    repsT = persist.tile([128, KC, FULL_TILES, 128], BF16)
    lhsT = persist.tile([128, KC, M_TILES, 128], BF16)

    # p-major staging: raw_full[:, t, :]: t in 0..31 -> emb_i row 32p+t,
    # t in 32..63 -> emb_j row 32p+(t-32). raw_my[:, m, :] -> my row 8p+m.
    raw_full = persist.tile([128, FULL_TILES, D], BF16)
    raw_my = persist.tile([128, M_TILES, D], BF16)
    rn_full = persist.tile([128, FULL_TILES, D], BF16)
    rn_my = persist.tile([128, M_TILES, D], BF16)

    NT = FULL_TILES + M_TILES  # 72 row-tiles to normalize (my first)
    ss_all = persist.tile([128, NT], F32)
    inv_all = persist.tile([128, NT], F32)
    pos_stage = persist.tile([128, 32], F32)
    diag_stage = persist.tile([128, M_TILES], F32)
    den_all = persist.tile([128, 32], F32)
    fin = persist.tile([128, 2], F32)

    # ---- loads (SWDGE casts f32 -> bf16 in flight; p-major = one big
    # contiguous chunk per partition per DMA) ----
    ei = emb_i.ap().rearrange("(p t) d -> p t d", p=128)  # [128, 32, 256]
    ej = emb_j.ap().rearrange("(p t) d -> p t d", p=128)
    mr = my_rows.ap().rearrange("(p t) d -> p t d", p=128)  # [128, 8, 256]
    nc.gpsimd.dma_start(out=raw_my[:, :, :], in_=mr)
    for h in range(2):
        nc.gpsimd.dma_start(
            out=raw_full[:, 16 * h:16 * (h + 1), :], in_=ei[:, 16 * h:16 * (h + 1), :])
    for h in range(2):
        nc.gpsimd.dma_start(
            out=raw_full[:, 32 + 16 * h:32 + 16 * (h + 1), :],
            in_=ej[:, 16 * h:16 * (h + 1), :])

    def src(i):
        # normalize-order i -> (raw tile AP, rn tile AP)
        if i < M_TILES:
            return raw_my[:, i, :], rn_my[:, i, :]
        return raw_full[:, i - M_TILES, :], rn_full[:, i - M_TILES, :]

    # ---- all row sum-squares, then ONE Ln + ONE Exp, then normalize ----
    for i in range(NT):
        r, _ = src(i)
        junk = work.tile([128, D], BF16, tag="sqjunk")
        nc.vector.scalar_tensor_tensor(
            out=junk[:, :], in0=r, scalar=1.0, in1=r,
            op0=ALU.bypass, op1=ALU.mult, accum_out=ss_all[:, i:i + 1])
    lnss = persist.tile([128, NT], F32)
    nc.scalar.activation(out=lnss[:, :], in_=ss_all[:, :], func=ACT.Ln)
    nc.scalar.activation(out=inv_all[:, :], in_=lnss[:, :], func=ACT.Exp, scale=-0.5)

    for i in range(NT):
        r, rn = src(i)
        nc.vector.tensor_scalar(
            out=rn, in0=r, scalar1=inv_all[:, i:i + 1], scalar2=None, op0=ALU.mult)

    # diag logits for my rows: 2*||rho_r||^2 (emitted early; feeds one Exp)
    for m in range(M_TILES):
        junk = work.tile([128, D], BF16, tag="sqjunk")
        nc.vector.scalar_tensor_tensor(
            out=junk[:, :], in0=rn_my[:, m, :], scalar=2.0, in1=rn_my[:, m, :],
            op0=ALU.mult, op1=ALU.mult, accum_out=diag_stage[:, m:m + 1])
    ediag = persist.tile([128, M_TILES], F32)
    nc.scalar.activation(out=ediag[:, :], in_=diag_stage[:, :], func=ACT.Exp)

    # positive term: 4 * dot(rho_i_r, rho_j_r) per pair
    for t in range(32):
        junk = work.tile([128, D], BF16, tag="sqjunk")
        nc.vector.scalar_tensor_tensor(
            out=junk[:, :], in0=rn_full[:, t, :], scalar=4.0,
            in1=rn_full[:, t + 32, :],
            op0=ALU.mult, op1=ALU.mult, accum_out=pos_stage[:, t:t + 1])

    # ---- transposes via DMA xbar (bf16 SBUF->SBUF), my rows first ----
    for m in range(M_TILES):
        for kc in range(KC):
            nc.sync.dma_start_transpose(
                out=lhsT[:, kc, m, :], in_=rn_my[:, m, kc * 128:(kc + 1) * 128])
    for t in range(FULL_TILES):
        for kc in range(KC):
            nc.sync.dma_start_transpose(
                out=repsT[:, kc, t, :], in_=rn_full[:, t, kc * 128:(kc + 1) * 128])

    # ---- main: G row-block in [128, 2048] psum tiles, exp+rowsum ----
    for m in range(M_TILES):
        for q in range(4):
            ps = psum_mm.tile([128, 2048], F32, tag="mm")
            for half in range(4):
                tb = q * 16 + half * 4
                for kc in range(KC):
                    nc.tensor.matmul(
                        out=ps[:, half * 512:(half + 1) * 512],
                        lhsT=lhsT[:, kc, m, :],
                        rhs=repsT[:, kc, tb:tb + 4, :],
                        start=(kc == 0), stop=(kc == 1))
            ej_ = work.tile([128, 2048], F32, tag="expjunk")
            nc.scalar.activation(
                out=ej_[:, :], in_=ps[:, :], func=ACT.Exp, scale=2.0,
                accum_out=den_all[:, m * 4 + q:m * 4 + q + 1])

    # denominators: [128, 8, 4] -> [128, 8], minus ediag, one batched Ln
    den8 = persist.tile([128, M_TILES], F32)
    nc.vector.tensor_reduce(
        out=den8[:, :], in_=den_all[:, :].rearrange("p (m q) -> p m q", q=4),
        axis=AXX.X, op=ALU.add)
    dex8 = persist.tile([128, M_TILES], F32)
    nc.vector.tensor_sub(dex8[:, :], den8[:, :], ediag[:, :])
    ld8 = persist.tile([128, M_TILES], F32)
    nc.scalar.activation(out=ld8[:, :], in_=dex8[:, :], func=ACT.Ln)

    nc.vector.tensor_reduce(out=fin[:, 0:1], in_=ld8[:, :], axis=AXX.X, op=ALU.add)
    nc.vector.tensor_reduce(out=fin[:, 1:2], in_=pos_stage[:, :], axis=AXX.X, op=ALU.add)
    nc.sync.dma_start(out=out.ap(), in_=fin[:, :])


_CACHED = None


def _build():
    global _CACHED
    if _CACHED is not None:
        return _CACHED
    nc = bacc.Bacc("TRN2", target_bir_lowering=False, debug=False,
                   enable_asserts=False, num_devices=N_CORES)
    emb_i = nc.dram_tensor("emb_i", [N, D], F32, kind="ExternalInput")
    emb_j = nc.dram_tensor("emb_j", [N, D], F32, kind="ExternalInput")
    my_rows = nc.dram_tensor("my_rows", [ROWS_PER_CORE, D], F32, kind="ExternalInput")
    out = nc.dram_tensor("out", [128, 2], F32, kind="ExternalOutput")
    with tile.TileContext(nc) as tc:
        with ExitStack() as ctx:
            _emit(nc, tc, ctx, emb_i, emb_j, my_rows, out)
    nc.compile()
    _CACHED = nc
    return nc


LAST_EXEC_NS = None
LAST_TRACE = None


def kernel(emb_i, emb_j, batch_size):
    global LAST_EXEC_NS, LAST_TRACE
    emb_i = np.ascontiguousarray(np.asarray(emb_i), dtype=np.float32)
    emb_j = np.ascontiguousarray(np.asarray(emb_j), dtype=np.float32)
    assert emb_i.shape == (N, D) and emb_j.shape == (N, D)
    concat = np.concatenate([emb_i, emb_j], axis=0)

    nc = _build()
    in_maps = []
    for c in range(N_CORES):
        in_maps.append({
            "emb_i": emb_i,
            "emb_j": emb_j,
            "my_rows": np.ascontiguousarray(
                concat[c * ROWS_PER_CORE:(c + 1) * ROWS_PER_CORE]),
        })
    trace = bool(int(os.environ.get("KERNEL_TRACE", "0")))
    res = run_bass_kernel_spmd(nc, in_maps, list(range(N_CORES)), trace=trace)
    LAST_EXEC_NS = res.exec_time_ns
    if res.instructions_and_trace is not None:
        LAST_TRACE = res.instructions_and_trace[1]

    total = 0.0
    for c in range(N_CORES):
        o = np.asarray(res.results[c]["out"], dtype=np.float64)
        total += o[:, 0].sum() - 0.125 * o[:, 1].sum()
    return np.array(total / TWO_N, dtype=np.float32)


# revision 8
# speedup vs baseline: 1.8763x; 1.8763x over previous
"""Trainium2 Bass kernel for NT-Xent contrastive loss (N=4096, D=256).

loss = mean_i(log(sum_{k!=i} exp(sim(r_i,r_k)/T)) - sim(r_i, r_{i+N mod 2N})/T)
with r = row-l2-normalized concat(emb_i, emb_j), T = 0.5.

Sharding: rows of the [8192, 8192] similarity matrix are split across the
8 cores (1024 rows each, passed per-core as `my_rows`). Every core builds
the full normalized transposed reps [256, 8192] (bf16) in SBUF via DMA
xbar transposes, computes its row-block of the Gram matrix on the PE in
[128, 2048] psum tiles, does exp+row-sum on the Scalar engine (fused
accumulator), excludes the diagonal analytically (exp(2*||rho_r||^2)),
takes one batched log, and reduces. Ln/Exp activations are batched so the
ACT table set never thrashes. The positive term is computed from
normalized row pairs on the Vector engine (identical on every core; each
core subtracts 1/8 of it). Host sums the 8 [128, 2] partials.
"""

import os
import numpy as np

import concourse.bass as bass
import concourse.bacc as bacc
import concourse.tile as tile
from concourse import mybir
from concourse.bass_utils import run_bass_kernel_spmd
from contextlib import ExitStack

N = 4096
D = 256
TWO_N = 2 * N
N_CORES = 8
ROWS_PER_CORE = TWO_N // N_CORES  # 1024
M_TILES = ROWS_PER_CORE // 128    # 8
FULL_TILES = TWO_N // 128         # 64 (32 from emb_i, 32 from emb_j)
KC = 2                            # 256 = 2 chunks of 128 on partitions

F32 = mybir.dt.float32
BF16 = mybir.dt.bfloat16
ALU = mybir.AluOpType
ACT = mybir.ActivationFunctionType
AXX = mybir.AxisListType


def _emit(nc, tc, ctx, emb_i, emb_j, my_rows, out):
    persist = ctx.enter_context(tc.tile_pool(name="persist", bufs=1))
    work = ctx.enter_context(tc.tile_pool(name="work", bufs=3))
    psum_mm = ctx.enter_context(tc.tile_pool(name="psum_mm", bufs=2, space="PSUM"))

    # ---- persistent SBUF ----
    repsT = persist.tile([128, KC, FULL_TILES, 128], BF16)
    lhsT = persist.tile([128, KC, M_TILES, 128], BF16)

    # p-major staging: raw_full[:, t, :]: t in 0..31 -> emb_i row 32p+t,
    # t in 32..63 -> emb_j row 32p+(t-32). raw_my[:, m, :] -> my row 8p+m.
    raw_full = persist.tile([128, FULL_TILES, D], BF16)
    raw_my = persist.tile([128, M_TILES, D], BF16)
    rn_full = persist.tile([128, FULL_TILES, D], BF16)
    rn_my = persist.tile([128, M_TILES, D], BF16)

    NT = FULL_TILES + M_TILES  # 72 row-tiles to normalize (my first)
    ss_all = persist.tile([128, NT], F32)
    inv_all = persist.tile([128, NT], F32)
    pos_stage = persist.tile([128, 32], F32)
    diag_stage = persist.tile([128, M_TILES], F32)
    den_all = persist.tile([128, 32], F32)
    fin = persist.tile([128, 2], F32)

    # ---- loads (SWDGE casts f32 -> bf16 in flight; p-major = one big
    # contiguous chunk per partition per DMA) ----
    ei = emb_i.ap().rearrange("(p t) d -> p t d", p=128)  # [128, 32, 256]
    ej = emb_j.ap().rearrange("(p t) d -> p t d", p=128)
    mr = my_rows.ap().rearrange("(p t) d -> p t d", p=128)  # [128, 8, 256]
    nc.gpsimd.dma_start(out=raw_my[:, :, :], in_=mr)
    for h in range(2):
        nc.gpsimd.dma_start(
            out=raw_full[:, 16 * h:16 * (h + 1), :], in_=ei[:, 16 * h:16 * (h + 1), :])
    for h in range(2):
        nc.gpsimd.dma_start(
            out=raw_full[:, 32 + 16 * h:32 + 16 * (h + 1), :],
            in_=ej[:, 16 * h:16 * (h + 1), :])

    def src(i):
        # normalize-order i -> (raw tile AP, rn tile AP)
        if i < M_TILES:
            return raw_my[:, i, :], rn_my[:, i, :]
        return raw_full[:, i - M_TILES, :], rn_full[:, i - M_TILES, :]

    # ---- all row sum-squares, then ONE Ln + ONE Exp, then normalize ----
    for i in range(NT):
        r, _ = src(i)
        junk = work.tile([128, D], BF16, tag="sqjunk")
        nc.vector.scalar_tensor_tensor(
            out=junk[:, :], in0=r, scalar=1.0, in1=r,
            op0=ALU.bypass, op1=ALU.mult, accum_out=ss_all[:, i:i + 1])
    lnss = persist.tile([128, NT], F32)
    nc.scalar.activation(out=lnss[:, :], in_=ss_all[:, :], func=ACT.Ln)
    nc.scalar.activation(out=inv_all[:, :], in_=lnss[:, :], func=ACT.Exp, scale=-0.5)

    for i in range(NT):
        r, rn = src(i)
        nc.vector.tensor_scalar(
            out=rn, in0=r, scalar1=inv_all[:, i:i + 1], scalar2=None, op0=ALU.mult)

    # diag logits for my rows: 2*||rho_r||^2 (emitted early; feeds one Exp)
    for m in range(M_TILES):
        junk = work.tile([128, D], BF16, tag="sqjunk")
        nc.vector.scalar_tensor_tensor(
            out=junk[:, :], in0=rn_my[:, m, :], scalar=2.0, in1=rn_my[:, m, :],
            op0=ALU.mult, op1=ALU.mult, accum_out=diag_stage[:, m:m + 1])
    ediag = persist.tile([128, M_TILES], F32)
    nc.scalar.activation(out=ediag[:, :], in_=diag_stage[:, :], func=ACT.Exp)

    # positive term: 4 * dot(rho_i_r, rho_j_r) per pair
    for t in range(32):
        junk = work.tile([128, D], BF16, tag="sqjunk")
        nc.vector.scalar_tensor_tensor(
            out=junk[:, :], in0=rn_full[:, t, :], scalar=4.0,
            in1=rn_full[:, t + 32, :],
            op0=ALU.mult, op1=ALU.mult, accum_out=pos_stage[:, t:t + 1])

    # ---- transposes on the PE (bf16, identity matmul), batched 4 row-tiles
    # (8 [128,128] blocks) per 1-bank psum tile, one DVE evac each.
    # psum tiles share the "mm" pool slots; transposes drain before the
    # matmuls need the slots.
    ident = persist.tile([128, 128], BF16)
    from concourse.masks import make_identity
    make_identity(nc, ident)

    def transpose4(rn, t0, ntile, dstT, d0):
        ps = psum_mm.tile([128, 2 * ntile, 128], BF16, tag="mm")
        for j in range(ntile):
            for kc in range(KC):
                nc.tensor.transpose(
                    out=ps[:, 2 * j + kc, :],
                    in_=rn[:, t0 + j, kc * 128:(kc + 1) * 128],
                    identity=ident[:, :])
        nc.vector.tensor_copy(
            dstT[:, :, d0:d0 + ntile, :].rearrange("p kc t c -> p t kc c"),
            ps[:, :, :].rearrange("p (t kc) c -> p t kc c", kc=KC))

    for m in range(0, M_TILES, 4):
        transpose4(rn_my, m, 4, lhsT, m)
    for t in range(0, FULL_TILES, 4):
        transpose4(rn_full, t, 4, repsT, t)

    # ---- main: G row-block in [128, 2048] psum tiles, exp+rowsum ----
    for m in range(M_TILES):
        for q in range(4):
            ps = psum_mm.tile([128, 2048], F32, tag="mm")
            for half in range(4):
                tb = q * 16 + half * 4
                for kc in range(KC):
                    nc.tensor.matmul(
                        out=ps[:, half * 512:(half + 1) * 512],
                        lhsT=lhsT[:, kc, m, :],
                        rhs=repsT[:, kc, tb:tb + 4, :],
                        start=(kc == 0), stop=(kc == 1))
            ej_ = work.tile([128, 2048], F32, tag="expjunk")
            nc.scalar.activation(
                out=ej_[:, :], in_=ps[:, :], func=ACT.Exp, scale=2.0,
                accum_out=den_all[:, m * 4 + q:m * 4 + q + 1])

    # denominators: [128, 8, 4] -> [128, 8], minus ediag, one batched Ln
    den8 = persist.tile([128, M_TILES], F32)
    nc.vector.tensor_reduce(
        out=den8[:, :], in_=den_all[:, :].rearrange("p (m q) -> p m q", q=4),
        axis=AXX.X, op=ALU.add)
    dex8 = persist.tile([128, M_TILES], F32)
    nc.vector.tensor_sub(dex8[:, :], den8[:, :], ediag[:, :])
    ld8 = persist.tile([128, M_TILES], F32)
    nc.scalar.activation(out=ld8[:, :], in_=dex8[:, :], func=ACT.Ln)

    nc.vector.tensor_reduce(out=fin[:, 0:1], in_=ld8[:, :], axis=AXX.X, op=ALU.add)
    nc.vector.tensor_reduce(out=fin[:, 1:2], in_=pos_stage[:, :], axis=AXX.X, op=ALU.add)
    nc.sync.dma_start(out=out.ap(), in_=fin[:, :])


_CACHED = None


def _build():
    global _CACHED
    if _CACHED is not None:
        return _CACHED
    nc = bacc.Bacc("TRN2", target_bir_lowering=False, debug=False,
                   enable_asserts=False, num_devices=N_CORES)
    emb_i = nc.dram_tensor("emb_i", [N, D], F32, kind="ExternalInput")
    emb_j = nc.dram_tensor("emb_j", [N, D], F32, kind="ExternalInput")
    my_rows = nc.dram_tensor("my_rows", [ROWS_PER_CORE, D], F32, kind="ExternalInput")
    out = nc.dram_tensor("out", [128, 2], F32, kind="ExternalOutput")
    with tile.TileContext(nc) as tc:
        with ExitStack() as ctx:
            _emit(nc, tc, ctx, emb_i, emb_j, my_rows, out)
    nc.compile()
    _CACHED = nc
    return nc


LAST_EXEC_NS = None
LAST_TRACE = None


def kernel(emb_i, emb_j, batch_size):
    global LAST_EXEC_NS, LAST_TRACE
    emb_i = np.ascontiguousarray(np.asarray(emb_i), dtype=np.float32)
    emb_j = np.ascontiguousarray(np.asarray(emb_j), dtype=np.float32)
    assert emb_i.shape == (N, D) and emb_j.shape == (N, D)
    concat = np.concatenate([emb_i, emb_j], axis=0)

    nc = _build()
    in_maps = []
    for c in range(N_CORES):
        in_maps.append({
            "emb_i": emb_i,
            "emb_j": emb_j,
            "my_rows": np.ascontiguousarray(
                concat[c * ROWS_PER_CORE:(c + 1) * ROWS_PER_CORE]),
        })
    trace = bool(int(os.environ.get("KERNEL_TRACE", "0")))
    res = run_bass_kernel_spmd(nc, in_maps, list(range(N_CORES)), trace=trace)
    LAST_EXEC_NS = res.exec_time_ns
    if res.instructions_and_trace is not None:
        LAST_TRACE = res.instructions_and_trace[1]

    total = 0.0
    for c in range(N_CORES):
        o = np.asarray(res.results[c]["out"], dtype=np.float64)
        total += o[:, 0].sum() - 0.125 * o[:, 1].sum()
    return np.array(total / TWO_N, dtype=np.float32)


# revision 9
# speedup vs baseline: 2.0838x; 1.1106x over previous
"""Trainium2 Bass kernel for NT-Xent contrastive loss (N=4096, D=256).

loss = mean_i(log(sum_{k!=i} exp(sim(r_i,r_k)/T)) - sim(r_i, r_{i+N mod 2N})/T)
with r = row-l2-normalized concat(emb_i, emb_j), T = 0.5.

Sharding: rows of the [8192, 8192] similarity matrix are split across the
8 cores (1024 rows each, passed per-core as `my_rows`). Every core builds
the full normalized transposed reps [256, 8192] (bf16) in SBUF (PE
identity transposes), computes its row-block of the Gram matrix on the PE
in [128, 2048] psum tiles, does exp+row-sum on the Scalar engine (fused
accumulator), excludes the diagonal analytically (exp(2*||rho_r||^2)),
takes one batched log, and reduces. The transpose work is phase-
interleaved with the ACT-bound main loop so it hides completely; Ln/Exp
activations are batched so the ACT table set never thrashes. The positive
term is computed from normalized row pairs on the Vector engine during
the main loop (identical on every core; each core subtracts 1/8 of it).
Host sums the 8 [128, 2] partials.
"""

import os
import numpy as np

import concourse.bass as bass
import concourse.bacc as bacc
import concourse.tile as tile
from concourse import mybir
from concourse.bass_utils import run_bass_kernel_spmd
from concourse.masks import make_identity
from contextlib import ExitStack

N = 4096
D = 256
TWO_N = 2 * N
N_CORES = 8
ROWS_PER_CORE = TWO_N // N_CORES  # 1024
M_TILES = ROWS_PER_CORE // 128    # 8
FULL_TILES = TWO_N // 128         # 64 (32 from emb_i, 32 from emb_j)
KC = 2                            # 256 = 2 chunks of 128 on partitions

F32 = mybir.dt.float32
BF16 = mybir.dt.bfloat16
ALU = mybir.AluOpType
ACT = mybir.ActivationFunctionType
AXX = mybir.AxisListType


def _emit(nc, tc, ctx, emb_i, emb_j, my_rows, out):
    persist = ctx.enter_context(tc.tile_pool(name="persist", bufs=1))
    work = ctx.enter_context(tc.tile_pool(name="work", bufs=3))
    psum_mm = ctx.enter_context(tc.tile_pool(name="psum_mm", bufs=2, space="PSUM"))

    # ---- persistent SBUF ----
    repsT = persist.tile([128, KC, FULL_TILES, 128], BF16)
    lhsT = persist.tile([128, KC, M_TILES, 128], BF16)
    ident = persist.tile([128, 128], BF16)
    make_identity(nc, ident)

    # p-major staging: raw_full[:, t, :]: t in 0..31 -> emb_i row 32p+t,
    # t in 32..63 -> emb_j row 32p+(t-32). raw_my[:, m, :] -> my row 8p+m.
    raw_full = persist.tile([128, FULL_TILES, D], BF16)
    raw_my = persist.tile([128, M_TILES, D], BF16)
    rn_full = persist.tile([128, FULL_TILES, D], BF16)
    rn_my = persist.tile([128, M_TILES, D], BF16)

    ss_my = persist.tile([128, M_TILES], F32)
    inv_my = persist.tile([128, M_TILES], F32)
    ss_full = persist.tile([128, FULL_TILES], F32)
    inv_full = persist.tile([128, FULL_TILES], F32)
    pos_stage = persist.tile([128, 32], F32)
    diag_stage = persist.tile([128, M_TILES], F32)
    den_all = persist.tile([128, 32], F32)
    fin = persist.tile([128, 2], F32)

    # ---- loads (SWDGE casts f32 -> bf16 in flight; p-major = one big
    # contiguous chunk per partition per DMA); my rows first ----
    ei = emb_i.ap().rearrange("(p t) d -> p t d", p=128)  # [128, 32, 256]
    ej = emb_j.ap().rearrange("(p t) d -> p t d", p=128)
    mr = my_rows.ap().rearrange("(p t) d -> p t d", p=128)  # [128, 8, 256]
    nc.gpsimd.dma_start(out=raw_my[:, :, :], in_=mr)
    for h in range(2):
        nc.gpsimd.dma_start(
            out=raw_full[:, 16 * h:16 * (h + 1), :], in_=ei[:, 16 * h:16 * (h + 1), :])
    for h in range(2):
        nc.gpsimd.dma_start(
            out=raw_full[:, 32 + 16 * h:32 + 16 * (h + 1), :],
            in_=ej[:, 16 * h:16 * (h + 1), :])

    def squares(raw, t, ss_ap):
        junk = work.tile([128, D], BF16, tag="sqjunk")
        nc.vector.scalar_tensor_tensor(
            out=junk[:, :], in0=raw, scalar=1.0, in1=raw,
            op0=ALU.bypass, op1=ALU.mult, accum_out=ss_ap)

    def transpose_group(rn, t0, ntile, dstT, d0):
        ps = psum_mm.tile([128, 2 * ntile, 128], BF16, tag="mm")
        for j in range(ntile):
            for kc in range(KC):
                nc.tensor.transpose(
                    out=ps[:, 2 * j + kc, :],
                    in_=rn[:, t0 + j, kc * 128:(kc + 1) * 128],
                    identity=ident[:, :])
        nc.vector.tensor_copy(
            dstT[:, :, d0:d0 + ntile, :].rearrange("p kc t c -> p t kc c"),
            ps[:, :, :].rearrange("p (t kc) c -> p t kc c", kc=KC))

    # ---- my rows mini-pipeline: lhsT ready ASAP ----
    for m in range(M_TILES):
        squares(raw_my[:, m, :], m, ss_my[:, m:m + 1])
    lnss_my = persist.tile([128, M_TILES], F32)
    nc.scalar.activation(out=lnss_my[:, :], in_=ss_my[:, :], func=ACT.Ln)
    nc.scalar.activation(out=inv_my[:, :], in_=lnss_my[:, :], func=ACT.Exp,
                         scale=-0.5)
    for m in range(M_TILES):
        nc.vector.tensor_scalar(
            out=rn_my[:, m, :], in0=raw_my[:, m, :], scalar1=inv_my[:, m:m + 1],
            scalar2=None, op0=ALU.mult)
    transpose_group(rn_my, 0, 4, lhsT, 0)
    transpose_group(rn_my, 4, 4, lhsT, 4)

    # ---- full squares (overlaps the tail of the loads), one Ln+Exp ----
    for t in range(FULL_TILES):
        squares(raw_full[:, t, :], t, ss_full[:, t:t + 1])
    lnss_f = persist.tile([128, FULL_TILES], F32)
    nc.scalar.activation(out=lnss_f[:, :], in_=ss_full[:, :], func=ACT.Ln)
    nc.scalar.activation(out=inv_full[:, :], in_=lnss_f[:, :], func=ACT.Exp,
                         scale=-0.5)

    # ---- phase-interleaved: normalize+transpose 16 column-tiles, then the
    # 8 [128, 2048] Gram tiles that consume them. Transposes of phase k+1
    # hide under the ACT-bound exp of phase k. ----
    for k in range(4):
        for g in range(4):
            t0 = 16 * k + 4 * g
            for j in range(4):
                t = t0 + j
                nc.vector.tensor_scalar(
                    out=rn_full[:, t, :], in0=raw_full[:, t, :],
                    scalar1=inv_full[:, t:t + 1], scalar2=None, op0=ALU.mult)
            transpose_group(rn_full, t0, 4, repsT, t0)
        for m in range(M_TILES):
            ps = psum_mm.tile([128, 2048], F32, tag="mm")
            for kc in range(KC):
                for half in range(4):
                    tb = k * 16 + half * 4
                    nc.tensor.matmul(
                        out=ps[:, half * 512:(half + 1) * 512],
                        lhsT=lhsT[:, kc, m, :],
                        rhs=repsT[:, kc, tb:tb + 4, :],
                        start=(kc == 0), stop=(kc == 1))
            ej_ = work.tile([128, 2048], F32, tag="expjunk")
            nc.scalar.activation(
                out=ej_[:, :], in_=ps[:, :], func=ACT.Exp, scale=2.0,
                accum_out=den_all[:, m * 4 + k:m * 4 + k + 1])

    # ---- positive + diag terms: DVE is idle during the ACT-bound main
    # loop, so these are emitted last and fill the gaps. ----
    for t in range(32):
        junk = work.tile([128, D], BF16, tag="sqjunk")
        nc.vector.scalar_tensor_tensor(
            out=junk[:, :], in0=rn_full[:, t, :], scalar=4.0,
            in1=rn_full[:, t + 32, :],
            op0=ALU.mult, op1=ALU.mult, accum_out=pos_stage[:, t:t + 1])
    for m in range(M_TILES):
        junk = work.tile([128, D], BF16, tag="sqjunk")
        nc.vector.scalar_tensor_tensor(
            out=junk[:, :], in0=rn_my[:, m, :], scalar=2.0, in1=rn_my[:, m, :],
            op0=ALU.mult, op1=ALU.mult, accum_out=diag_stage[:, m:m + 1])
    ediag = persist.tile([128, M_TILES], F32)
    nc.scalar.activation(out=ediag[:, :], in_=diag_stage[:, :], func=ACT.Exp)

    # denominators: [128, 8, 4] -> [128, 8], minus ediag, one batched Ln
    den8 = persist.tile([128, M_TILES], F32)
    nc.vector.tensor_reduce(
        out=den8[:, :], in_=den_all[:, :].rearrange("p (m q) -> p m q", q=4),
        axis=AXX.X, op=ALU.add)
    dex8 = persist.tile([128, M_TILES], F32)
    nc.vector.tensor_sub(dex8[:, :], den8[:, :], ediag[:, :])
    ld8 = persist.tile([128, M_TILES], F32)
    nc.scalar.activation(out=ld8[:, :], in_=dex8[:, :], func=ACT.Ln)

    nc.vector.tensor_reduce(out=fin[:, 0:1], in_=ld8[:, :], axis=AXX.X, op=ALU.add)
    nc.vector.tensor_reduce(out=fin[:, 1:2], in_=pos_stage[:, :], axis=AXX.X, op=ALU.add)
    nc.sync.dma_start(out=out.ap(), in_=fin[:, :])


_CACHED = None


def _build():
    global _CACHED
    if _CACHED is not None:
        return _CACHED
    nc = bacc.Bacc("TRN2", target_bir_lowering=False, debug=False,
                   enable_asserts=False, num_devices=N_CORES)
    emb_i = nc.dram_tensor("emb_i", [N, D], F32, kind="ExternalInput")
    emb_j = nc.dram_tensor("emb_j", [N, D], F32, kind="ExternalInput")
    my_rows = nc.dram_tensor("my_rows", [ROWS_PER_CORE, D], F32, kind="ExternalInput")
    out = nc.dram_tensor("out", [128, 2], F32, kind="ExternalOutput")
    with tile.TileContext(nc) as tc:
        with ExitStack() as ctx:
            _emit(nc, tc, ctx, emb_i, emb_j, my_rows, out)
    nc.compile()
    _CACHED = nc
    return nc


LAST_EXEC_NS = None
LAST_TRACE = None


def kernel(emb_i, emb_j, batch_size):
    global LAST_EXEC_NS, LAST_TRACE
    emb_i = np.ascontiguousarray(np.asarray(emb_i), dtype=np.float32)
    emb_j = np.ascontiguousarray(np.asarray(emb_j), dtype=np.float32)
    assert emb_i.shape == (N, D) and emb_j.shape == (N, D)
    concat = np.concatenate([emb_i, emb_j], axis=0)

    nc = _build()
    in_maps = []
    for c in range(N_CORES):
        in_maps.append({
            "emb_i": emb_i,
            "emb_j": emb_j,
            "my_rows": np.ascontiguousarray(
                concat[c * ROWS_PER_CORE:(c + 1) * ROWS_PER_CORE]),
        })
    trace = bool(int(os.environ.get("KERNEL_TRACE", "0")))
    res = run_bass_kernel_spmd(nc, in_maps, list(range(N_CORES)), trace=trace)
    LAST_EXEC_NS = res.exec_time_ns
    if res.instructions_and_trace is not None:
        LAST_TRACE = res.instructions_and_trace[1]

    total = 0.0
    for c in range(N_CORES):
        o = np.asarray(res.results[c]["out"], dtype=np.float64)
        total += o[:, 0].sum() - 0.125 * o[:, 1].sum()
    return np.array(total / TWO_N, dtype=np.float32)
